# revision 28
# baseline (speedup 1.0000x reference)
"""Trainium2 Bass kernel for the CoAttention scoring layer.

reference:
    keys    = receiver @ w_k                      # [B, R, D]
    queries = attendant @ w_q                     # [B, A, D]
    e_act   = queries[:, None, :, :] + keys[:, :, None, :] + bias  # [B, R, A, D]
    out     = tanh(e_act) @ a                     # [B, R, A]

Sharding: data-parallel over B across 8 NeuronCores (8 batches per core),
params replicated.

Host prep: inputs are pre-transposed to (F, R)/(F, A) layout and cast to
fp16 (device DMA cannot cast, and fp16 matmuls are single-pass on the PE
while fp32 lowers to LOW/HIGH double passes).

Per-core device layout (D=128 in the partition dim):
    kT (D, R), qT (D, A), k_nat (R, D), q_nat (A, D)  via fp16 matmuls
    e chunks (D, CA*R): e[d, j, r] = qbT[d, a0+j] + kT[d, r]
        - most chunks: one DVE broadcast tensor_add (SBUF)
        - some chunks: PE identity-selector matmuls into PSUM
    th = tanh(e) on ACT (fp16 out; PSUM-sourced chunks fold bias as the
        per-partition activation bias)
    scores[:, a] = th_a^T @ a_vec: per-a fp16 matvec, (R, A) PSUM tile
    scores -> SBUF -> DMA per batch.
"""

import sys

if "/opt/trn_rl_repo" not in sys.path:
    sys.path.insert(0, "/opt/trn_rl_repo")

from contextlib import ExitStack

import numpy as np

import concourse.bacc as bacc
import concourse.bass as bass
import concourse.tile as tile
from concourse import masks, mybir
from concourse.bass_utils import run_bass_kernel_spmd

B, R, A, F = 64, 128, 128, 256
D = F // 2
NCORES = 8
BC = B // NCORES  # batches per core
CA = 32           # a-indices per chunk (free dim = CA * R = 4096)
NCHUNK = A // CA  # chunks per batch
PE_CHUNKS = (0,)  # chunk indices whose broadcast-add runs on the PE
EP_FD = 512       # PSUM e-tile free dim (one bank; fp32 matmul-out limit)
JE = EP_FD // R   # a-indices per PSUM e-tile
F32 = mybir.dt.float32
F16 = mybir.dt.float16
BF16 = mybir.dt.bfloat16

_CACHE = {}


def build_bass():
    nc = bacc.Bacc("TRN2", target_bir_lowering=False, debug=False)

    rT_d = nc.declare_dram_parameter("receiverT16", [BC, F, R], F16, isOutput=False)
    aT_d = nc.declare_dram_parameter("attendantT16", [BC, F, A], F16, isOutput=False)
    wq_d = nc.declare_dram_parameter("w_q16", [F, D], F16, isOutput=False)
    wk_d = nc.declare_dram_parameter("w_k16", [F, D], F16, isOutput=False)
    bias_d = nc.declare_dram_parameter("bias", [D, 1], F32, isOutput=False)
    a_d = nc.declare_dram_parameter("a16", [D, 1], BF16, isOutput=False)
    selq_d = nc.declare_dram_parameter("selq16", [A, len(PE_CHUNKS) * CA * R], F16, isOutput=False)
    selr_d = nc.declare_dram_parameter("selr16", [R, JE * R], F16, isOutput=False)
    out = nc.declare_dram_parameter("out", [BC, R, A], F32, isOutput=True)

    TANH = mybir.ActivationFunctionType.Tanh

    with tile.TileContext(nc) as tc, ExitStack() as ctx:
        const = ctx.enter_context(tc.tile_pool(name="const", bufs=1))
        natp = ctx.enter_context(tc.tile_pool(name="nat", bufs=3))
        kqp = ctx.enter_context(tc.tile_pool(name="kqp", bufs=2, space="PSUM"))
        kqs = ctx.enter_context(tc.tile_pool(name="kqs", bufs=2))
        ep = ctx.enter_context(tc.tile_pool(name="ep", bufs=3))
        epp = ctx.enter_context(tc.tile_pool(name="epp", bufs=2, space="PSUM"))
        thp = ctx.enter_context(tc.tile_pool(name="thp", bufs=3))
        scp = ctx.enter_context(tc.tile_pool(name="scp", bufs=2, space="PSUM"))
        scs = ctx.enter_context(tc.tile_pool(name="scs", bufs=2))

        # constants (selectors go on a different DMA queue: off the warmup path)
        selq_sb = const.tile([A, len(PE_CHUNKS) * CA * R], F16, tag="selq")
        nc.scalar.dma_start(selq_sb[:], selq_d[:])
        selr_sb = const.tile([R, JE * R], F16, tag="selr")
        nc.scalar.dma_start(selr_sb[:], selr_d[:])
        wk_sb = const.tile([128, F], F16, tag="wk")   # [f_local, ft*D + d]
        wq_sb = const.tile([128, F], F16, tag="wq")
        for ft in range(2):
            nc.sync.dma_start(wk_sb[:, ft * D:(ft + 1) * D], wk_d[ft * 128:(ft + 1) * 128])
            nc.sync.dma_start(wq_sb[:, ft * D:(ft + 1) * D], wq_d[ft * 128:(ft + 1) * 128])
        bias_col = const.tile([D, 1], F32, tag="bias")
        nc.sync.dma_start(bias_col[:], bias_d[:])
        a_f16 = const.tile([D, 1], BF16, tag="avec16")
        nc.sync.dma_start(a_f16[:], a_d[:])

        for b in range(BC):
            # transposed fp16 inputs: [f_local, ft*128 + r]
            rT = natp.tile([128, F], F16, tag="rT")
            aT = natp.tile([128, F], F16, tag="aT")
            for ft in range(2):
                nc.sync.dma_start(rT[:, ft * 128:(ft + 1) * 128], rT_d[b, ft * 128:(ft + 1) * 128])
                nc.sync.dma_start(aT[:, ft * 128:(ft + 1) * 128], aT_d[b, ft * 128:(ft + 1) * 128])

            # kT = w_k^T @ recv^T (D, R); k_nat = recv @ w_k (R, D); same for q
            kT_ps = kqp.tile([D, R], F32, tag="kq")
            nc.tensor.matmul(kT_ps[:], wk_sb[:, 0:D], rT[:, 0:128], start=True, stop=False)
            nc.tensor.matmul(kT_ps[:], wk_sb[:, D:2 * D], rT[:, 128:256], start=False, stop=True)
            kT_sb = kqs.tile([D, R], F32, tag="kT_sb")
            nc.vector.tensor_copy(kT_sb[:], kT_ps[:])

            qT_ps = kqp.tile([D, A], F32, tag="kq")
            nc.tensor.matmul(qT_ps[:], wq_sb[:, 0:D], aT[:, 0:128], start=True, stop=False)
            nc.tensor.matmul(qT_ps[:], wq_sb[:, D:2 * D], aT[:, 128:256], start=False, stop=True)
            qbT_sb = kqs.tile([D, A], F32, tag="qbT_sb")
            nc.vector.tensor_scalar_add(qbT_sb[:], qT_ps[:], bias_col[:, 0:1])

            kn_ps = kqp.tile([R, D], F32, tag="kq")
            nc.tensor.matmul(kn_ps[:], rT[:, 0:128], wk_sb[:, 0:D], start=True, stop=False)
            nc.tensor.matmul(kn_ps[:], rT[:, 128:256], wk_sb[:, D:2 * D], start=False, stop=True)
            k_nat = kqs.tile([R, D], F16, tag="k_nat")
            nc.vector.tensor_copy(k_nat[:], kn_ps[:])

            qn_ps = kqp.tile([A, D], F32, tag="kq")
            nc.tensor.matmul(qn_ps[:], aT[:, 0:128], wq_sb[:, 0:D], start=True, stop=False)
            nc.tensor.matmul(qn_ps[:], aT[:, 128:256], wq_sb[:, D:2 * D], start=False, stop=True)
            q_nat = kqs.tile([A, D], F16, tag="q_nat")
            nc.vector.tensor_copy(q_nat[:], qn_ps[:])

            sc_ps = scp.tile([R, A], F32, tag="sc_ps")
            for ac in range(NCHUNK):
                a0 = ac * CA
                th = thp.tile([D, CA, R], BF16, tag="th")
                if ac in PE_CHUNKS:
                    # PE path: e[d, (j, r)] = q_nat^T selectA + k_nat^T selectR
                    # in 512-col PSUM slices; ACT folds +bias during tanh.
                    pci = PE_CHUNKS.index(ac)
                    NSL = 1024 // EP_FD  # matmul slices per psum tile
                    j8 = 1024 // R       # a-indices per psum tile
                    for s in range(CA * R // 1024):
                        e_ps = epp.tile([D, 1024], F32, tag="e_ps")
                        for t in range(NSL):
                            o = (pci * CA // JE + s * NSL + t) * EP_FD
                            nc.tensor.matmul(
                                e_ps[:, t * EP_FD:(t + 1) * EP_FD],
                                q_nat[:],
                                selq_sb[:, o:o + EP_FD],
                                start=True,
                                stop=False,
                            )
                            nc.tensor.matmul(
                                e_ps[:, t * EP_FD:(t + 1) * EP_FD],
                                k_nat[:],
                                selr_sb[:],
                                start=False,
                                stop=True,
                            )
                        nc.scalar.activation(
                            th[:, s * j8:(s + 1) * j8], e_ps[:], TANH, bias=bias_col[:, 0:1]
                        )
                else:
                    # DVE path: one broadcast add, then one big tanh
                    e = ep.tile([D, CA, R], F32, tag="e")
                    in0 = qbT_sb[:, a0:a0 + CA].unsqueeze(2).broadcast_to([D, CA, R])
                    in1 = kT_sb[:].unsqueeze(1).broadcast_to([D, CA, R])
                    nc.vector.tensor_add(e[:], in0, in1)
                    nc.scalar.activation(th[:], e[:], TANH)
                for j in range(CA):
                    nc.tensor.matmul(
                        sc_ps[:, a0 + j:a0 + j + 1],
                        th[:, j],
                        a_f16[:],
                        start=True,
                        stop=True,
                    )
            sc_sb = scs.tile([R, A], F32, tag="sc_sb")
            nc.vector.tensor_copy(sc_sb[:], sc_ps[:])
            nc.sync.dma_start(out[b], sc_sb[:])

    nc.finalize()
    return nc


def _get_nc():
    if "nc" not in _CACHE:
        _CACHE["nc"] = build_bass()
    return _CACHE["nc"]


def make_in_maps(inputs):
    receiver = np.ascontiguousarray(inputs["receiver"], dtype=np.float32)
    attendant = np.ascontiguousarray(inputs["attendant"], dtype=np.float32)
    w_q16 = np.ascontiguousarray(inputs["w_q"], dtype=np.float16)
    w_k16 = np.ascontiguousarray(inputs["w_k"], dtype=np.float16)
    bias = np.ascontiguousarray(inputs["bias"], dtype=np.float32).reshape(D, 1)
    import ml_dtypes
    a16 = np.ascontiguousarray(inputs["a"], dtype=ml_dtypes.bfloat16).reshape(D, 1)
    recvT16 = np.ascontiguousarray(receiver.transpose(0, 2, 1).astype(np.float16))
    attnT16 = np.ascontiguousarray(attendant.transpose(0, 2, 1).astype(np.float16))
    # selector constants for the PE broadcast-add chunks
    eye = np.eye(A, dtype=np.float16)
    selq = np.zeros((A, len(PE_CHUNKS), CA, R), dtype=np.float16)
    for i, pc in enumerate(PE_CHUNKS):
        selq[:, i, :, :] = eye[:, pc * CA:(pc + 1) * CA][:, :, None]
    selq16 = np.ascontiguousarray(selq.reshape(A, -1))
    selr16 = np.ascontiguousarray(
        np.broadcast_to(eye[:, None, :], (R, JE, R)).reshape(R, -1).astype(np.float16)
    )
    in_maps = []
    for c in range(NCORES):
        in_maps.append(
            {
                "receiverT16": np.ascontiguousarray(recvT16[c * BC:(c + 1) * BC]),
                "attendantT16": np.ascontiguousarray(attnT16[c * BC:(c + 1) * BC]),
                "w_q16": w_q16,
                "w_k16": w_k16,
                "bias": bias,
                "a16": a16,
                "selq16": selq16,
                "selr16": selr16,
            }
        )
    return in_maps


def run(inputs, **kwargs):
    nc = _get_nc()
    in_maps = make_in_maps(inputs)
    res = run_bass_kernel_spmd(nc, in_maps, list(range(NCORES)), **kwargs)
    out = np.concatenate([res.results[c]["out"] for c in range(NCORES)], axis=0)
    return out, res


def kernel(**inputs) -> np.ndarray:
    out, _ = run(inputs)
    return out


# revision 31
# speedup vs baseline: 1.0132x; 1.0132x over previous
"""Trainium2 Bass kernel for the CoAttention scoring layer.

reference:
    keys    = receiver @ w_k                      # [B, R, D]
    queries = attendant @ w_q                     # [B, A, D]
    e_act   = queries[:, None, :, :] + keys[:, :, None, :] + bias  # [B, R, A, D]
    out     = tanh(e_act) @ a                     # [B, R, A]

Sharding: data-parallel over B across 8 NeuronCores (8 batches per core),
params replicated.

Host prep: inputs are pre-transposed to (F, R)/(F, A) layout and cast to
fp16 (device DMA cannot cast, and fp16 matmuls are single-pass on the PE
while fp32 lowers to LOW/HIGH double passes).

Per-core device layout (D=128 in the partition dim):
    kT (D, R), qT (D, A), k_nat (R, D), q_nat (A, D)  via fp16 matmuls
    e chunks (D, CA*R): e[d, j, r] = qbT[d, a0+j] + kT[d, r]
        - most chunks: one DVE broadcast tensor_add (SBUF)
        - some chunks: PE identity-selector matmuls into PSUM
    th = tanh(e) on ACT (fp16 out; PSUM-sourced chunks fold bias as the
        per-partition activation bias)
    scores[:, a] = th_a^T @ a_vec: per-a fp16 matvec, (R, A) PSUM tile
    scores -> SBUF -> DMA per batch.
"""

import sys

if "/opt/trn_rl_repo" not in sys.path:
    sys.path.insert(0, "/opt/trn_rl_repo")

from contextlib import ExitStack

import numpy as np

import concourse.bacc as bacc
import concourse.bass as bass
import concourse.tile as tile
from concourse import masks, mybir
from concourse.bass_utils import run_bass_kernel_spmd

B, R, A, F = 64, 128, 128, 256
D = F // 2
NCORES = 8
BC = B // NCORES  # batches per core
CA = 32           # a-indices per chunk (free dim = CA * R = 4096)
NCHUNK = A // CA  # chunks per batch
PE_CHUNKS = (3,)  # chunk indices whose broadcast-add runs on the PE
EP_FD = 512       # PSUM e-tile free dim (one bank; fp32 matmul-out limit)
JE = EP_FD // R   # a-indices per PSUM e-tile
F32 = mybir.dt.float32
F16 = mybir.dt.float16
BF16 = mybir.dt.bfloat16

_CACHE = {}


def build_bass():
    nc = bacc.Bacc("TRN2", target_bir_lowering=False, debug=False)

    rT_d = nc.declare_dram_parameter("receiverT16", [BC, F, R], F16, isOutput=False)
    aT_d = nc.declare_dram_parameter("attendantT16", [BC, F, A], F16, isOutput=False)
    wq_d = nc.declare_dram_parameter("w_q16", [F, D], F16, isOutput=False)
    wk_d = nc.declare_dram_parameter("w_k16", [F, D], F16, isOutput=False)
    bias_d = nc.declare_dram_parameter("bias", [D, 1], F32, isOutput=False)
    a_d = nc.declare_dram_parameter("a16", [D, 1], F16, isOutput=False)
    selq_d = nc.declare_dram_parameter("selq16", [A, len(PE_CHUNKS) * CA * R], F16, isOutput=False)
    selr_d = nc.declare_dram_parameter("selr16", [R, JE * R], F16, isOutput=False)
    out = nc.declare_dram_parameter("out", [BC, R, A], F32, isOutput=True)

    TANH = mybir.ActivationFunctionType.Tanh

    with tile.TileContext(nc) as tc, ExitStack() as ctx:
        const = ctx.enter_context(tc.tile_pool(name="const", bufs=1))
        natp = ctx.enter_context(tc.tile_pool(name="nat", bufs=3))
        kqp = ctx.enter_context(tc.tile_pool(name="kqp", bufs=2, space="PSUM"))
        kqs = ctx.enter_context(tc.tile_pool(name="kqs", bufs=2))
        ep = ctx.enter_context(tc.tile_pool(name="ep", bufs=3))
        epp = ctx.enter_context(tc.tile_pool(name="epp", bufs=2, space="PSUM"))
        thp = ctx.enter_context(tc.tile_pool(name="thp", bufs=3))
        scp = ctx.enter_context(tc.tile_pool(name="scp", bufs=2, space="PSUM"))
        scs = ctx.enter_context(tc.tile_pool(name="scs", bufs=2))

        # constants (selectors go on a different DMA queue: off the warmup path)
        selq_sb = const.tile([A, len(PE_CHUNKS) * CA * R], F16, tag="selq")
        nc.scalar.dma_start(selq_sb[:], selq_d[:])
        selr_sb = const.tile([R, JE * R], F16, tag="selr")
        nc.scalar.dma_start(selr_sb[:], selr_d[:])
        wk_sb = const.tile([128, F], F16, tag="wk")   # [f_local, ft*D + d]
        wq_sb = const.tile([128, F], F16, tag="wq")
        for ft in range(2):
            nc.sync.dma_start(wk_sb[:, ft * D:(ft + 1) * D], wk_d[ft * 128:(ft + 1) * 128])
            nc.sync.dma_start(wq_sb[:, ft * D:(ft + 1) * D], wq_d[ft * 128:(ft + 1) * 128])
        bias_col = const.tile([D, 1], F32, tag="bias")
        nc.sync.dma_start(bias_col[:], bias_d[:])
        a_f16 = const.tile([D, 1], F16, tag="avec16")
        nc.sync.dma_start(a_f16[:], a_d[:])

        for b in range(BC):
            # transposed fp16 inputs: [f_local, ft*128 + r]
            rT = natp.tile([128, F], F16, tag="rT")
            aT = natp.tile([128, F], F16, tag="aT")
            for ft in range(2):
                nc.sync.dma_start(rT[:, ft * 128:(ft + 1) * 128], rT_d[b, ft * 128:(ft + 1) * 128])
                nc.sync.dma_start(aT[:, ft * 128:(ft + 1) * 128], aT_d[b, ft * 128:(ft + 1) * 128])

            # kT = w_k^T @ recv^T (D, R); k_nat = recv @ w_k (R, D); same for q
            kT_ps = kqp.tile([D, R], F32, tag="kq")
            nc.tensor.matmul(kT_ps[:], wk_sb[:, 0:D], rT[:, 0:128], start=True, stop=False)
            nc.tensor.matmul(kT_ps[:], wk_sb[:, D:2 * D], rT[:, 128:256], start=False, stop=True)
            kT_sb = kqs.tile([D, R], F32, tag="kT_sb")
            nc.vector.tensor_copy(kT_sb[:], kT_ps[:])

            qT_ps = kqp.tile([D, A], F32, tag="kq")
            nc.tensor.matmul(qT_ps[:], wq_sb[:, 0:D], aT[:, 0:128], start=True, stop=False)
            nc.tensor.matmul(qT_ps[:], wq_sb[:, D:2 * D], aT[:, 128:256], start=False, stop=True)
            qbT_sb = kqs.tile([D, A], F32, tag="qbT_sb")
            nc.vector.tensor_scalar_add(qbT_sb[:], qT_ps[:], bias_col[:, 0:1])

            kn_ps = kqp.tile([R, D], F32, tag="kq")
            nc.tensor.matmul(kn_ps[:], rT[:, 0:128], wk_sb[:, 0:D], start=True, stop=False)
            nc.tensor.matmul(kn_ps[:], rT[:, 128:256], wk_sb[:, D:2 * D], start=False, stop=True)
            k_nat = kqs.tile([R, D], F16, tag="k_nat")
            nc.vector.tensor_copy(k_nat[:], kn_ps[:])

            qn_ps = kqp.tile([A, D], F32, tag="kq")
            nc.tensor.matmul(qn_ps[:], aT[:, 0:128], wq_sb[:, 0:D], start=True, stop=False)
            nc.tensor.matmul(qn_ps[:], aT[:, 128:256], wq_sb[:, D:2 * D], start=False, stop=True)
            q_nat = kqs.tile([A, D], F16, tag="q_nat")
            nc.vector.tensor_copy(q_nat[:], qn_ps[:])

            sc_ps = scp.tile([R, A], F32, tag="sc_ps")
            for ac in range(NCHUNK):
                a0 = ac * CA
                th = thp.tile([D, CA, R], F16, tag="th")
                if ac in PE_CHUNKS:
                    # PE path: e[d, (j, r)] = q_nat^T selectA + k_nat^T selectR
                    # in 512-col PSUM slices; ACT folds +bias during tanh.
                    pci = PE_CHUNKS.index(ac)
                    NSL = 1024 // EP_FD  # matmul slices per psum tile
                    j8 = 1024 // R       # a-indices per psum tile
                    for s in range(CA * R // 1024):
                        e_ps = epp.tile([D, 1024], F32, tag="e_ps")
                        for t in range(NSL):
                            o = (pci * CA // JE + s * NSL + t) * EP_FD
                            nc.tensor.matmul(
                                e_ps[:, t * EP_FD:(t + 1) * EP_FD],
                                q_nat[:],
                                selq_sb[:, o:o + EP_FD],
                                start=True,
                                stop=False,
                            )
                            nc.tensor.matmul(
                                e_ps[:, t * EP_FD:(t + 1) * EP_FD],
                                k_nat[:],
                                selr_sb[:],
                                start=False,
                                stop=True,
                            )
                        nc.scalar.activation(
                            th[:, s * j8:(s + 1) * j8], e_ps[:], TANH, bias=bias_col[:, 0:1]
                        )
                else:
                    # DVE path: broadcast add(s), then tanh. The very first and
                    # last chunks of the kernel run in quarter pieces so the
                    # ACT pipeline ramps up / drains with less idle time.
                    e = ep.tile([D, CA, R], F32, tag="e")
                    ramp = (b == 0 and ac == 0) or (b == BC - 1 and ac == NCHUNK - 2)
                    CS = CA // 4 if ramp else CA
                    for c0 in range(0, CA, CS):
                        in0 = (
                            qbT_sb[:, a0 + c0:a0 + c0 + CS]
                            .unsqueeze(2)
                            .broadcast_to([D, CS, R])
                        )
                        in1 = kT_sb[:].unsqueeze(1).broadcast_to([D, CS, R])
                        nc.vector.tensor_add(e[:, c0:c0 + CS], in0, in1)
                        nc.scalar.activation(th[:, c0:c0 + CS], e[:, c0:c0 + CS], TANH)
                for j in range(CA):
                    nc.tensor.matmul(
                        sc_ps[:, a0 + j:a0 + j + 1],
                        th[:, j],
                        a_f16[:],
                        start=True,
                        stop=True,
                    )
            sc_sb = scs.tile([R, A], F32, tag="sc_sb")
            nc.vector.tensor_copy(sc_sb[:], sc_ps[:])
            nc.sync.dma_start(out[b], sc_sb[:])

    nc.finalize()
    return nc


def _get_nc():
    if "nc" not in _CACHE:
        _CACHE["nc"] = build_bass()
    return _CACHE["nc"]


def make_in_maps(inputs):
    receiver = np.ascontiguousarray(inputs["receiver"], dtype=np.float32)
    attendant = np.ascontiguousarray(inputs["attendant"], dtype=np.float32)
    w_q16 = np.ascontiguousarray(inputs["w_q"], dtype=np.float16)
    w_k16 = np.ascontiguousarray(inputs["w_k"], dtype=np.float16)
    bias = np.ascontiguousarray(inputs["bias"], dtype=np.float32).reshape(D, 1)
    a16 = np.ascontiguousarray(inputs["a"], dtype=np.float16).reshape(D, 1)
    recvT16 = np.ascontiguousarray(receiver.transpose(0, 2, 1).astype(np.float16))
    attnT16 = np.ascontiguousarray(attendant.transpose(0, 2, 1).astype(np.float16))
    # selector constants for the PE broadcast-add chunks
    eye = np.eye(A, dtype=np.float16)
    selq = np.zeros((A, len(PE_CHUNKS), CA, R), dtype=np.float16)
    for i, pc in enumerate(PE_CHUNKS):
        selq[:, i, :, :] = eye[:, pc * CA:(pc + 1) * CA][:, :, None]
    selq16 = np.ascontiguousarray(selq.reshape(A, -1))
    selr16 = np.ascontiguousarray(
        np.broadcast_to(eye[:, None, :], (R, JE, R)).reshape(R, -1).astype(np.float16)
    )
    in_maps = []
    for c in range(NCORES):
        in_maps.append(
            {
                "receiverT16": np.ascontiguousarray(recvT16[c * BC:(c + 1) * BC]),
                "attendantT16": np.ascontiguousarray(attnT16[c * BC:(c + 1) * BC]),
                "w_q16": w_q16,
                "w_k16": w_k16,
                "bias": bias,
                "a16": a16,
                "selq16": selq16,
                "selr16": selr16,
            }
        )
    return in_maps


def run(inputs, **kwargs):
    nc = _get_nc()
    in_maps = make_in_maps(inputs)
    res = run_bass_kernel_spmd(nc, in_maps, list(range(NCORES)), **kwargs)
    out = np.concatenate([res.results[c]["out"] for c in range(NCORES)], axis=0)
    return out, res


def kernel(**inputs) -> np.ndarray:
    out, _ = run(inputs)
    return out
